# revision 14
# baseline (speedup 1.0000x reference)
"""Trainium2 Bass kernel for 2-layer BaseRGCN (basis decomposition).

Strategy (8 NeuronCores, SPMD — identical program, per-core data):
  - Relation sharding: core c owns relations {2c, 2c+1}. Host folds the basis
    decomposition into per-relation weight matrices W[r] = sum_b coef[r,b] V[b]
    and hands each core its pair.
  - Per layer: transform phase computes Hr[r] = h @ W[r] for the core's two
    relations into a DRAM table [2N, d] (h^T tiles as the matmul stationary);
    scatter phase gathers Hr[rel*N + src] rows per edge with dma_gather,
    builds a one-hot selection matrix O[e, n] = norm_e * (dst_rel_e == n) on
    the vector engine, and accumulates out^T[o, n] += G^T(e,o) @ O(e,n) into
    per-dst-window PSUM slices; window groups flush once to a partial
    [128, N] (transposed) buffer.
  - Partials are AllReduce'd across the 8 cores in 4 node-range chunks so the
    collectives overlap the remaining scatter work. ReLU+bias1 is fused into
    layer 2's transform input load. bias2 + final transpose happen on host.
  - Edges are sorted by dst-window inside 16-window chunks and split into 4
    int16 index ranges (dma_gather indices are int16; table has 2N = 100000
    rows, ranges of 25000); groups are padded to 32-slot quanta and packed
    into 128-slot tiles without crossing tile boundaries.
"""

import numpy as np

N_NODES = 50000
H_DIM = 128
NUM_RELS = 16
N_CORES = 8
RELS_PER_CORE = NUM_RELS // N_CORES
P = 128                      # partitions / window size / feature dim
W_TOT = (N_NODES + P - 1) // P          # 391 dst windows (last partial: 80)
CHUNK_W = 16                 # dst windows per scatter chunk (PSUM capacity)
N_CHUNKS = (W_TOT + CHUNK_W - 1) // CHUNK_W   # 25
RANGE = 25000                # int16 index range per dma_gather call
N_RANGES = 4                 # 2*N_NODES / RANGE
QUANT = 32                   # group padding quantum (PE row-group alignment)
AR_CHUNK_OF = [6, 6, 6, 7]   # scatter chunks per all-reduce chunk
DEBUG_STAGE = 4              # 1: transform only; 2: +scatter; 3: +AR; 4: full
SC_SUB = 4                   # scatter sub-stage: 1 gathers, 2 +onehot, 3 +matmul, 4 +flush

_CACHE = {}


def _build_edge_org(src, dst, etype, norm):
    """Host-side edge organization. Returns per-core arrays + uniform program
    metadata (identical across cores)."""
    rng_info = []
    # per-core edge fields
    core_edges = []
    for c in range(N_CORES):
        m = (etype // RELS_PER_CORE) == c
        s, d, e, nr = src[m], dst[m], etype[m] - c * RELS_PER_CORE, norm[m, 0]
        gidx = e.astype(np.int64) * N_NODES + s
        q = gidx // RANGE
        i16 = (gidx - q * RANGE).astype(np.int16)
        w = d // P
        drel = (d - w * P).astype(np.float32)
        core_edges.append((w, q, i16, drel, nr.astype(np.float32)))

    # unified per-(chunk, q, window) counts: max over cores (same program)
    counts = np.zeros((N_CORES, W_TOT, N_RANGES), np.int64)
    for c in range(N_CORES):
        w, q, _, _, _ = core_edges[c]
        np.add.at(counts[c], (w, q), 1)
    mx = counts.max(axis=0)                       # [W_TOT, N_RANGES]
    mx[:, 0] = np.maximum(mx[:, 0], 1)            # every window gets >=1 slot

    # tight packing: per (chunk, q): windows back-to-back, call padded to 128.
    # matmuls are full-128 per (tile-col, window) pair with window-masked
    # one-hots (no sub-partition matmuls - HW accumulation-group bug).
    prog = []
    chunk_cols = []
    slot_of = []        # per chunk: (q, w) -> chunk-local slot start
    pair_cols = 0
    for ch in range(N_CHUNKS):
        w0 = ch * CHUNK_W
        w1 = min(w0 + CHUNK_W, W_TOT)
        gslot = {}
        call_cols = []
        col_base = 0
        pairs = []        # (col_local, w_local) in slot order
        for q in range(N_RANGES):
            pos = 0
            for w in range(w0, w1):
                cnt = int(mx[w, q])
                if cnt == 0:
                    continue
                gslot[(q, w)] = col_base * P + pos
                c_lo = col_base + pos // P
                c_hi = col_base + (pos + cnt - 1) // P
                for cc in range(c_lo, c_hi + 1):
                    pairs.append((cc, w - w0))
                pos += cnt
            qc = (pos + P - 1) // P
            call_cols.append(qc)
            col_base += qc
        # window-major pair ordering with first/last flags
        win_pairs = {}
        for (cc, wl) in pairs:
            win_pairs.setdefault(wl, []).append(cc)
        wblocks = []
        for wl in sorted(win_pairs):
            lst = sorted(set(win_pairs[wl]))
            wblocks.append((wl, lst))
        n_pairs = sum(len(lst) for _, lst in wblocks)
        prog.append(dict(call_cols=call_cols, wblocks=wblocks,
                         n_cols=col_base, n_pairs=n_pairs, w0=w0, w1=w1,
                         pair_base=pair_cols))
        slot_of.append(gslot)
        chunk_cols.append(col_base)
        pair_cols += n_pairs
        assert col_base <= 64, f"chunk {ch}: {col_base} cols"

    tot_cols = int(np.sum(chunk_cols))
    chunk_slab_off = np.concatenate([[0], np.cumsum(chunk_cols)]) * P

    # per-core slot arrays
    idx16 = np.zeros((N_CORES, tot_cols * P), np.int16)
    slot_w = np.full(tot_cols * P, -1, np.int64)       # slot -> global window
    drel_slot = np.full((N_CORES, tot_cols * P), -1.0, np.float32)
    norm_slot = np.zeros((N_CORES, tot_cols * P), np.float32)
    for c in range(N_CORES):
        w, q, i16, drel, nr = core_edges[c]
        ch = w // CHUNK_W
        order = np.lexsort((w, q, ch))
        cursor = {}
        for ei in order:
            key = (int(ch[ei]), int(q[ei]), int(w[ei]))
            ofs = cursor.get(key, 0)
            cursor[key] = ofs + 1
            slot = int(chunk_slab_off[key[0]]) + slot_of[key[0]][(key[1], key[2])] + ofs
            idx16[c, slot] = i16[ei]
            drel_slot[c, slot] = drel[ei]
            norm_slot[c, slot] = nr[ei]
            slot_w[slot] = key[2]

    # per-(pair) dstrel/norm columns: window-masked
    drel_2d = np.full((N_CORES, P, pair_cols), -1.0, np.float32)
    norm_2d = np.zeros((N_CORES, P, pair_cols), np.float32)
    pc = 0
    for ch in range(N_CHUNKS):
        pr = prog[ch]
        cbase = int(chunk_slab_off[ch]) // P
        pair_index = {}
        for (wl, lst) in pr["wblocks"]:
            for cc in lst:
                pair_index[(cc, wl)] = pc
                pc += 1
        for (cc, wl), pj in pair_index.items():
            s0 = (cbase + cc) * P
            wglob = pr["w0"] + wl
            for c in range(N_CORES):
                mask = slot_w[s0:s0 + P] == wglob
                drel_2d[c, mask, pj] = drel_slot[c, s0:s0 + P][mask]
                norm_2d[c, mask, pj] = norm_slot[c, s0:s0 + P][mask]
    assert pc == pair_cols

    # dma_gather idx arrays (wrap 16 + replicate to 128)
    gidx_2d = np.zeros((N_CORES, 128, tot_cols * P // 16), np.int16)
    for c in range(N_CORES):
        a = idx16[c].reshape(tot_cols * P // 16, 16).T
        gidx_2d[c] = np.tile(a, (8, 1))
    return prog, drel_2d, norm_2d, gidx_2d, chunk_cols, pair_cols


def _build_bass(prog, chunk_cols, pair_cols):
    from concourse import bacc, bass, mybir, tile

    f32 = mybir.dt.float32
    i16 = mybir.dt.int16
    nc = bacc.Bacc("TRN2", target_bir_lowering=False, debug=False,
                   num_devices=N_CORES, dynamic_dma_scratch_size=65536)

    tot_cols = int(np.sum(chunk_cols))
    hT = nc.dram_tensor("hT", [P, N_NODES], f32, kind="ExternalInput")
    Wp1 = nc.dram_tensor("Wp1", [RELS_PER_CORE, P, P], f32, kind="ExternalInput")
    Wp2 = nc.dram_tensor("Wp2", [RELS_PER_CORE, P, P], f32, kind="ExternalInput")
    iota = nc.dram_tensor("iota", [P, P], f32, kind="ExternalInput")
    bias1 = nc.dram_tensor("bias1", [P, 1], f32, kind="ExternalInput")
    drel = nc.dram_tensor("drel", [P, pair_cols], f32, kind="ExternalInput")
    normv = nc.dram_tensor("normv", [P, pair_cols], f32, kind="ExternalInput")
    gidx = nc.dram_tensor("gidx", [P, tot_cols * 8], i16, kind="ExternalInput")

    # AR chunk column extents
    ar_bounds = []
    chs = 0
    for nch in AR_CHUNK_OF:
        ch_lo, ch_hi = chs, chs + nch
        w_lo = ch_lo * CHUNK_W
        w_hi = min(ch_hi * CHUNK_W, W_TOT)
        c_lo = w_lo * P
        c_hi = min(w_hi * P, N_NODES)
        ar_bounds.append((ch_lo, ch_hi, c_lo, c_hi))
        chs = ch_hi

    Hr = [nc.dram_tensor(f"Hr{l}", [2 * N_NODES, P], f32) for l in (0, 1)]
    partial = [[nc.dram_tensor(f"pt{l}_{a}", [P, b[3] - b[2]], f32)
                for a, b in enumerate(ar_bounds)] for l in (0, 1)]
    red = [[nc.dram_tensor(f"rd{l}_{a}", [P, b[3] - b[2]], f32,
                           addr_space="Shared")
            for a, b in enumerate(ar_bounds)] for l in (0, 1)]
    outs = [nc.dram_tensor(f"out{a}", [P, b[3] - b[2]], f32,
                           kind="ExternalOutput")
            for a, b in enumerate(ar_bounds)]

    chunk_slab_col = np.concatenate([[0], np.cumsum(chunk_cols)]).astype(int)

    with tile.TileContext(nc) as tc:
        with tc.tile_pool(name="const", bufs=1) as constp:
            iota_t = constp.tile([P, P], f32)
            nc.sync.dma_start(out=iota_t[:], in_=iota[:])
            bias1_t = constp.tile([P, 1], f32)
            nc.sync.dma_start(out=bias1_t[:], in_=bias1[:])
            drel_t = constp.tile([P, pair_cols], f32)
            nc.sync.dma_start(out=drel_t[:], in_=drel[:])
            norm_t = constp.tile([P, pair_cols], f32)
            nc.sync.dma_start(out=norm_t[:], in_=normv[:])
            gidx_t = constp.tile([P, tot_cols * 8], i16)
            nc.sync.dma_start(out=gidx_t[:], in_=gidx[:])
            w1_t = constp.tile([P, RELS_PER_CORE * P], f32)
            for r in range(RELS_PER_CORE):
                nc.sync.dma_start(out=w1_t[:, r * P:(r + 1) * P], in_=Wp1[r])
            w2_t = constp.tile([P, RELS_PER_CORE * P], f32)
            for r in range(RELS_PER_CORE):
                nc.sync.dma_start(out=w2_t[:, r * P:(r + 1) * P], in_=Wp2[r])

            def transform(layer, w_t):
                """Compute Hr[layer] from h^T source. layer 0: hT input;
                layer 1: relu(red[0] + bias1)."""
                with tc.tile_pool(name=f"tf{layer}", bufs=3) as sb, \
                     tc.tile_pool(name=f"tfp{layer}", bufs=4, space="PSUM") as pp, \
                     tc.tile_pool(name=f"tfs{layer}", bufs=3) as stp:
                    for ch in range(N_CHUNKS):
                        w0 = ch * CHUNK_W
                        w1 = min(w0 + CHUNK_W, W_TOT)
                        c_lo = w0 * P
                        c_hi = min(w1 * P, N_NODES)
                        ncols = c_hi - c_lo
                        ht_c = sb.tile([P, CHUNK_W * P], f32, tag="ht")
                        if layer == 0:
                            nc.sync.dma_start(out=ht_c[:, :ncols],
                                              in_=hT[:, c_lo:c_hi])
                            src_c = ht_c
                        else:
                            # find AR chunk containing this scatter chunk
                            for a, (al, ah, acl, ach) in enumerate(ar_bounds):
                                if al <= ch < ah:
                                    break
                            nc.sync.dma_start(
                                out=ht_c[:, :ncols],
                                in_=red[0][a][:, c_lo - acl:c_hi - acl])
                            act_c = sb.tile([P, CHUNK_W * P], f32, tag="act")
                            nc.scalar.activation(
                                out=act_c[:, :ncols], in_=ht_c[:, :ncols],
                                func=mybir.ActivationFunctionType.Relu,
                                bias=bias1_t[:, :1], scale=1.0)
                            src_c = act_c
                        for wl in range(w1 - w0):
                            nt0 = (w0 + wl) * P
                            nrow = min(P, N_NODES - nt0)
                            ps = pp.tile([P, RELS_PER_CORE * P], f32,
                                         space="PSUM", tag="ps")
                            for r in range(RELS_PER_CORE):
                                nc.tensor.matmul(
                                    out=ps[:nrow, r * P:(r + 1) * P],
                                    lhsT=src_c[:, wl * P: wl * P + nrow],
                                    rhs=w_t[:, r * P:(r + 1) * P],
                                    start=True, stop=True)
                            st = stp.tile([P, RELS_PER_CORE * P], f32, tag="st")
                            nc.vector.tensor_copy(out=st[:nrow, :], in_=ps[:nrow, :])
                            for r in range(RELS_PER_CORE):
                                nc.sync.dma_start(
                                    out=Hr[layer][r * N_NODES + nt0:
                                                  r * N_NODES + nt0 + nrow, :],
                                    in_=st[:nrow, r * P:(r + 1) * P])

            def scatter(layer):
                """Gather Hr rows per edge, one-hot matmul into window PSUM,
                flush per chunk into partial[layer], AR per AR-chunk."""
                with tc.tile_pool(name=f"sc{layer}", bufs=2) as slabp, \
                     tc.tile_pool(name=f"oh{layer}", bufs=12) as ohp, \
                     tc.tile_pool(name=f"pw{layer}", bufs=6, space="PSUM") as pwp, \
                     tc.tile_pool(name=f"fl{layer}", bufs=2) as flp:
                    done_ar = 0
                    for ch in range(N_CHUNKS):
                        pr = prog[ch]
                        ncol = pr["n_cols"]
                        if ncol == 0:
                            continue
                        w0, w1 = pr["w0"], pr["w1"]
                        slab = slabp.tile([P, 64 * P], f32, tag="slab")
                        sl3 = slab[:].rearrange("p (t d) -> p t d", t=64)
                        cbase = int(chunk_slab_col[ch])
                        c0 = 0
                        if SC_SUB in (36, 37, 39):
                            nc.vector.memset(slab[:, :ncol * P], 0.25)
                        for q in range(N_RANGES):
                            qc = pr["call_cols"][q]
                            if qc == 0 or SC_SUB in (36, 37, 39):
                                continue
                            done = 0
                            while done < qc:
                                piece = min(8, qc - done)
                                nidx = piece * P
                                g0 = (cbase + c0 + done) * P // 16
                                nc.gpsimd.dma_gather(
                                    out_ap=sl3[:, c0 + done:c0 + done + piece, :],
                                    in_ap=Hr[layer][q * RANGE:, :],
                                    idxs_ap=gidx_t[:, g0:g0 + nidx // 16],
                                    num_idxs=nidx,
                                    num_idxs_reg=nidx,
                                    elem_size=P,
                                )
                                done += piece
                            c0 += qc
                        c_lo = w0 * P
                        c_hi = min(w1 * P, N_NODES)
                        nfc = c_hi - c_lo
                        fl = flp.tile([P, CHUNK_W * P], f32, tag="fl")
                        pj = pr["pair_base"]
                        for (wl, lst) in (pr["wblocks"] if SC_SUB >= 3 else []):
                            pw = pwp.tile([P, P], f32, space="PSUM", tag="pw")
                            for k, cc in enumerate(lst):
                                oh = ohp.tile([P, P], f32, tag="oh")
                                nc.vector.tensor_scalar(
                                    out=oh[:],
                                    in0=iota_t[:],
                                    scalar1=drel_t[:, pj: pj + 1],
                                    scalar2=norm_t[:, pj: pj + 1],
                                    op0=mybir.AluOpType.is_equal,
                                    op1=mybir.AluOpType.mult,
                                )
                                pj += 1
                                nc.tensor.matmul(
                                    out=pw[:],
                                    lhsT=slab[:, cc * P:(cc + 1) * P],
                                    rhs=oh[:],
                                    start=(k == 0), stop=(k == len(lst) - 1),
                                )
                            nwc = min(P, nfc - wl * P)
                            nc.vector.tensor_copy(out=fl[:, wl * P: wl * P + nwc],
                                                  in_=pw[:, :nwc])
                        if SC_SUB < 4:
                            continue
                        for a, (al, ah, acl, ach) in enumerate(ar_bounds):
                            if al <= ch < ah:
                                break
                        nc.sync.dma_start(
                            out=partial[layer][a][:, c_lo - acl:c_hi - acl],
                            in_=fl[:, :nfc])
                        # fire AR when its last chunk flushed
                        if DEBUG_STAGE >= 3 and ch + 1 == ar_bounds[done_ar][1]:
                            nc.gpsimd.collective_compute(
                                "AllReduce",
                                mybir.AluOpType.add,
                                ins=[partial[layer][done_ar].ap().opt()],
                                outs=[red[layer][done_ar].ap().opt()],
                                replica_groups=[list(range(N_CORES))],
                            )
                            done_ar += 1

            if SC_SUB not in (37, 39):
                transform(0, w1_t)
            if DEBUG_STAGE >= 2:
                scatter(0)
            if DEBUG_STAGE >= 3:
                transform(1, w2_t)
                scatter(1)
            if DEBUG_STAGE >= 4:
                for a in range(len(ar_bounds)):
                    nc.sync.dma_start(out=outs[a][:], in_=red[1][a][:])
            elif DEBUG_STAGE == 2:
                with tc.tile_pool(name="dbg", bufs=2) as dbp:
                    for a, (al, ah, acl, ach) in enumerate(ar_bounds):
                        for o in range(0, ach - acl, 2048):
                            oe = min(o + 2048, ach - acl)
                            t = dbp.tile([P, 2048], f32, tag="d")
                            nc.sync.dma_start(out=t[:, :oe - o],
                                              in_=partial[0][a][:, o:oe])
                            nc.sync.dma_start(out=outs[a][:, o:oe],
                                              in_=t[:, :oe - o])
            elif DEBUG_STAGE == 3:
                with tc.tile_pool(name="dbg", bufs=2) as dbp:
                    for a, (al, ah, acl, ach) in enumerate(ar_bounds):
                        for o in range(0, ach - acl, 2048):
                            oe = min(o + 2048, ach - acl)
                            t = dbp.tile([P, 2048], f32, tag="d")
                            nc.sync.dma_start(out=t[:, :oe - o],
                                              in_=red[1][a][:, o:oe])
                            nc.sync.dma_start(out=outs[a][:, o:oe],
                                              in_=t[:, :oe - o])

    nc.compile()
    return nc


def kernel(h, norm, src, dst, etype, V1, coef1, bias1, V2, coef2, bias2):
    import hashlib
    key = hashlib.md5(
        np.asarray(src).tobytes() + np.asarray(dst).tobytes()
        + np.asarray(etype).tobytes()
    ).hexdigest() + str(DEBUG_STAGE)
    if key not in _CACHE:
        prog, drel_2d, norm_2d, gidx_2d, chunk_cols, pair_cols = _build_edge_org(
            np.asarray(src), np.asarray(dst), np.asarray(etype), np.asarray(norm))
        nc = _build_bass(prog, chunk_cols, pair_cols)
        _CACHE[key] = (nc, drel_2d, norm_2d, gidx_2d)
    nc, drel_2d, norm_2d, gidx_2d = _CACHE[key]

    W1 = np.einsum("rb,bio->rio", np.asarray(coef1), np.asarray(V1)).astype(np.float32)
    W2 = np.einsum("rb,bio->rio", np.asarray(coef2), np.asarray(V2)).astype(np.float32)
    hT = np.ascontiguousarray(np.asarray(h).T).astype(np.float32)
    iota_np = np.broadcast_to(np.arange(P, dtype=np.float32), (P, P)).copy()
    b1 = np.asarray(bias1).astype(np.float32).reshape(P, 1)

    in_maps = []
    for c in range(N_CORES):
        in_maps.append({
            "hT": hT,
            "Wp1": np.ascontiguousarray(W1[2 * c:2 * c + 2]),
            "Wp2": np.ascontiguousarray(W2[2 * c:2 * c + 2]),
            "iota": iota_np,
            "bias1": b1,
            "drel": drel_2d[c],
            "normv": norm_2d[c],
            "gidx": gidx_2d[c],
        })

    from concourse.bass_utils import run_bass_kernel_spmd
    res = run_bass_kernel_spmd(nc, in_maps, core_ids=list(range(N_CORES)))
    r0 = res.results[0]
    outT = np.concatenate([r0[f"out{a}"] for a in range(len(AR_CHUNK_OF))], axis=1)
    out = outT.T + np.asarray(bias2).astype(np.float32)[None, :]
    return out.astype(np.float32)


# revision 15
# speedup vs baseline: 2367.2105x; 2367.2105x over previous
"""Trainium2 Bass kernel for 2-layer BaseRGCN (basis decomposition).

Strategy (8 NeuronCores, SPMD — identical program, per-core data):
  - Relation sharding: core c owns relations {2c, 2c+1}. Host folds the basis
    decomposition into per-relation weight matrices W[r] = sum_b coef[r,b] V[b]
    and hands each core its pair.
  - Per layer: transform phase computes Hr[r] = h @ W[r] for the core's two
    relations into a DRAM table [2N, d] (h^T tiles as the matmul stationary);
    scatter phase gathers Hr[rel*N + src] rows per edge with dma_gather,
    builds a one-hot selection matrix O[e, n] = norm_e * (dst_rel_e == n) on
    the vector engine, and accumulates out^T[o, n] += G^T(e,o) @ O(e,n) into
    per-dst-window PSUM slices; window groups flush once to a partial
    [128, N] (transposed) buffer.
  - Partials are AllReduce'd across the 8 cores in 4 node-range chunks so the
    collectives overlap the remaining scatter work. ReLU+bias1 is fused into
    layer 2's transform input load. bias2 + final transpose happen on host.
  - Edges are sorted by dst-window inside 16-window chunks and split into 4
    int16 index ranges (dma_gather indices are int16; table has 2N = 100000
    rows, ranges of 25000); groups are padded to 32-slot quanta and packed
    into 128-slot tiles without crossing tile boundaries.
"""

import numpy as np

N_NODES = 50000
H_DIM = 128
NUM_RELS = 16
N_CORES = 8
RELS_PER_CORE = NUM_RELS // N_CORES
P = 128                      # partitions / window size / feature dim
W_TOT = (N_NODES + P - 1) // P          # 391 dst windows (last partial: 80)
CHUNK_W = 16                 # dst windows per scatter chunk (PSUM capacity)
N_CHUNKS = (W_TOT + CHUNK_W - 1) // CHUNK_W   # 25
RANGE = 25000                # int16 index range per dma_gather call
N_RANGES = 4                 # 2*N_NODES / RANGE
QUANT = 32                   # group padding quantum (PE row-group alignment)
AR_CHUNK_OF = [6, 6, 6, 7]   # scatter chunks per all-reduce chunk
DEBUG_STAGE = 4              # 1: transform only; 2: +scatter; 3: +AR; 4: full
SC_SUB = 4                   # scatter sub-stage: 1 gathers, 2 +onehot, 3 +matmul, 4 +flush

_CACHE = {}
_LAST_IN_MAPS = None


def _build_edge_org(src, dst, etype, norm):
    """Host-side edge organization. Returns per-core arrays + uniform program
    metadata (identical across cores)."""
    rng_info = []
    # per-core edge fields
    core_edges = []
    for c in range(N_CORES):
        m = (etype // RELS_PER_CORE) == c
        s, d, e, nr = src[m], dst[m], etype[m] - c * RELS_PER_CORE, norm[m, 0]
        gidx = e.astype(np.int64) * N_NODES + s
        q = gidx // RANGE
        i16 = (gidx - q * RANGE).astype(np.int16)
        w = d // P
        drel = (d - w * P).astype(np.float32)
        core_edges.append((w, q, i16, drel, nr.astype(np.float32)))

    # unified per-(chunk, q, window) counts: max over cores (same program)
    counts = np.zeros((N_CORES, W_TOT, N_RANGES), np.int64)
    for c in range(N_CORES):
        w, q, _, _, _ = core_edges[c]
        np.add.at(counts[c], (w, q), 1)
    mx = counts.max(axis=0)                       # [W_TOT, N_RANGES]
    mx[:, 0] = np.maximum(mx[:, 0], 1)            # every window gets >=1 slot

    # tight packing: per (chunk, q): windows back-to-back, call padded to 128.
    # matmuls are full-128 per (tile-col, window) pair with window-masked
    # one-hots (no sub-partition matmuls - HW accumulation-group bug).
    prog = []
    chunk_cols = []
    slot_of = []        # per chunk: (q, w) -> chunk-local slot start
    pair_cols = 0
    for ch in range(N_CHUNKS):
        w0 = ch * CHUNK_W
        w1 = min(w0 + CHUNK_W, W_TOT)
        gslot = {}
        call_cols = []
        col_base = 0
        pairs = []        # (col_local, w_local) in slot order
        for q in range(N_RANGES):
            pos = 0
            for w in range(w0, w1):
                cnt = int(mx[w, q])
                if cnt == 0:
                    continue
                gslot[(q, w)] = col_base * P + pos
                c_lo = col_base + pos // P
                c_hi = col_base + (pos + cnt - 1) // P
                for cc in range(c_lo, c_hi + 1):
                    pairs.append((cc, w - w0))
                pos += cnt
            qc = (pos + P - 1) // P
            call_cols.append(qc)
            col_base += qc
        # window-major pair ordering with first/last flags
        win_pairs = {}
        for (cc, wl) in pairs:
            win_pairs.setdefault(wl, []).append(cc)
        wblocks = []
        for wl in sorted(win_pairs):
            lst = sorted(set(win_pairs[wl]))
            wblocks.append((wl, lst))
        n_pairs = sum(len(lst) for _, lst in wblocks)
        prog.append(dict(call_cols=call_cols, wblocks=wblocks,
                         n_cols=col_base, n_pairs=n_pairs, w0=w0, w1=w1,
                         pair_base=pair_cols))
        slot_of.append(gslot)
        chunk_cols.append(col_base)
        pair_cols += n_pairs
        assert col_base <= 64, f"chunk {ch}: {col_base} cols"

    tot_cols = int(np.sum(chunk_cols))
    chunk_slab_off = np.concatenate([[0], np.cumsum(chunk_cols)]) * P

    # per-core slot arrays
    idx16 = np.zeros((N_CORES, tot_cols * P), np.int16)
    slot_w = np.full(tot_cols * P, -1, np.int64)       # slot -> global window
    drel_slot = np.full((N_CORES, tot_cols * P), -1.0, np.float32)
    norm_slot = np.zeros((N_CORES, tot_cols * P), np.float32)
    for c in range(N_CORES):
        w, q, i16, drel, nr = core_edges[c]
        ch = w // CHUNK_W
        order = np.lexsort((w, q, ch))
        cursor = {}
        for ei in order:
            key = (int(ch[ei]), int(q[ei]), int(w[ei]))
            ofs = cursor.get(key, 0)
            cursor[key] = ofs + 1
            slot = int(chunk_slab_off[key[0]]) + slot_of[key[0]][(key[1], key[2])] + ofs
            idx16[c, slot] = i16[ei]
            drel_slot[c, slot] = drel[ei]
            norm_slot[c, slot] = nr[ei]
            slot_w[slot] = key[2]

    # per-(pair) dstrel/norm columns: window-masked
    drel_2d = np.full((N_CORES, P, pair_cols), -1.0, np.float32)
    norm_2d = np.zeros((N_CORES, P, pair_cols), np.float32)
    pc = 0
    for ch in range(N_CHUNKS):
        pr = prog[ch]
        cbase = int(chunk_slab_off[ch]) // P
        pair_index = {}
        for (wl, lst) in pr["wblocks"]:
            for cc in lst:
                pair_index[(cc, wl)] = pc
                pc += 1
        for (cc, wl), pj in pair_index.items():
            s0 = (cbase + cc) * P
            wglob = pr["w0"] + wl
            for c in range(N_CORES):
                mask = slot_w[s0:s0 + P] == wglob
                drel_2d[c, mask, pj] = drel_slot[c, s0:s0 + P][mask]
                norm_2d[c, mask, pj] = norm_slot[c, s0:s0 + P][mask]
    assert pc == pair_cols

    # dma_gather idx arrays (wrap 16 + replicate to 128)
    gidx_2d = np.zeros((N_CORES, 128, tot_cols * P // 16), np.int16)
    for c in range(N_CORES):
        a = idx16[c].reshape(tot_cols * P // 16, 16).T
        gidx_2d[c] = np.tile(a, (8, 1))
    return prog, drel_2d, norm_2d, gidx_2d, chunk_cols, pair_cols


def _build_bass(prog, chunk_cols, pair_cols):
    from concourse import bacc, bass, mybir, tile

    f32 = mybir.dt.float32
    i16 = mybir.dt.int16
    nc = bacc.Bacc("TRN2", target_bir_lowering=False, debug=False,
                   num_devices=N_CORES, dynamic_dma_scratch_size=65536)

    tot_cols = int(np.sum(chunk_cols))
    hT = nc.dram_tensor("hT", [P, N_NODES], f32, kind="ExternalInput")
    Wp1 = nc.dram_tensor("Wp1", [RELS_PER_CORE, P, P], f32, kind="ExternalInput")
    Wp2 = nc.dram_tensor("Wp2", [RELS_PER_CORE, P, P], f32, kind="ExternalInput")
    iota = nc.dram_tensor("iota", [P, P], f32, kind="ExternalInput")
    bias1 = nc.dram_tensor("bias1", [P, 1], f32, kind="ExternalInput")
    drel = nc.dram_tensor("drel", [P, pair_cols], f32, kind="ExternalInput")
    normv = nc.dram_tensor("normv", [P, pair_cols], f32, kind="ExternalInput")
    gidx = nc.dram_tensor("gidx", [P, tot_cols * 8], i16, kind="ExternalInput")

    # AR chunk column extents
    ar_bounds = []
    chs = 0
    for nch in AR_CHUNK_OF:
        ch_lo, ch_hi = chs, chs + nch
        w_lo = ch_lo * CHUNK_W
        w_hi = min(ch_hi * CHUNK_W, W_TOT)
        c_lo = w_lo * P
        c_hi = min(w_hi * P, N_NODES)
        ar_bounds.append((ch_lo, ch_hi, c_lo, c_hi))
        chs = ch_hi

    Hr = [nc.dram_tensor(f"Hr{l}", [2 * N_NODES, P], f32) for l in (0, 1)]
    partial = [[nc.dram_tensor(f"pt{l}_{a}", [P, b[3] - b[2]], f32)
                for a, b in enumerate(ar_bounds)] for l in (0, 1)]
    red = [[nc.dram_tensor(f"rd{l}_{a}", [P, b[3] - b[2]], f32,
                           addr_space="Shared")
            for a, b in enumerate(ar_bounds)] for l in (0, 1)]
    outs = [nc.dram_tensor(f"out{a}", [P, b[3] - b[2]], f32,
                           kind="ExternalOutput")
            for a, b in enumerate(ar_bounds)]

    chunk_slab_col = np.concatenate([[0], np.cumsum(chunk_cols)]).astype(int)

    with tile.TileContext(nc) as tc:
        with tc.tile_pool(name="const", bufs=1) as constp:
            iota_t = constp.tile([P, P], f32)
            nc.sync.dma_start(out=iota_t[:], in_=iota[:])
            bias1_t = constp.tile([P, 1], f32)
            nc.sync.dma_start(out=bias1_t[:], in_=bias1[:])
            drel_t = constp.tile([P, pair_cols], f32)
            nc.sync.dma_start(out=drel_t[:], in_=drel[:])
            norm_t = constp.tile([P, pair_cols], f32)
            nc.sync.dma_start(out=norm_t[:], in_=normv[:])
            gidx_t = constp.tile([P, tot_cols * 8], i16)
            nc.sync.dma_start(out=gidx_t[:], in_=gidx[:])
            w1_t = constp.tile([P, RELS_PER_CORE * P], f32)
            for r in range(RELS_PER_CORE):
                nc.sync.dma_start(out=w1_t[:, r * P:(r + 1) * P], in_=Wp1[r])
            w2_t = constp.tile([P, RELS_PER_CORE * P], f32)
            for r in range(RELS_PER_CORE):
                nc.sync.dma_start(out=w2_t[:, r * P:(r + 1) * P], in_=Wp2[r])

            def transform(layer, w_t):
                """Compute Hr[layer] from h^T source. layer 0: hT input;
                layer 1: relu(red[0] + bias1)."""
                with tc.tile_pool(name=f"tf{layer}", bufs=3) as sb, \
                     tc.tile_pool(name=f"tfp{layer}", bufs=4, space="PSUM") as pp, \
                     tc.tile_pool(name=f"tfs{layer}", bufs=3) as stp:
                    for ch in range(N_CHUNKS):
                        w0 = ch * CHUNK_W
                        w1 = min(w0 + CHUNK_W, W_TOT)
                        c_lo = w0 * P
                        c_hi = min(w1 * P, N_NODES)
                        ncols = c_hi - c_lo
                        ht_c = sb.tile([P, CHUNK_W * P], f32, tag="ht")
                        if layer == 0:
                            nc.sync.dma_start(out=ht_c[:, :ncols],
                                              in_=hT[:, c_lo:c_hi])
                            src_c = ht_c
                        else:
                            # find AR chunk containing this scatter chunk
                            for a, (al, ah, acl, ach) in enumerate(ar_bounds):
                                if al <= ch < ah:
                                    break
                            nc.sync.dma_start(
                                out=ht_c[:, :ncols],
                                in_=red[0][a][:, c_lo - acl:c_hi - acl])
                            act_c = sb.tile([P, CHUNK_W * P], f32, tag="act")
                            nc.scalar.activation(
                                out=act_c[:, :ncols], in_=ht_c[:, :ncols],
                                func=mybir.ActivationFunctionType.Relu,
                                bias=bias1_t[:, :1], scale=1.0)
                            src_c = act_c
                        for wl in range(w1 - w0):
                            nt0 = (w0 + wl) * P
                            nrow = min(P, N_NODES - nt0)
                            ps = pp.tile([P, RELS_PER_CORE * P], f32,
                                         space="PSUM", tag="ps")
                            for r in range(RELS_PER_CORE):
                                nc.tensor.matmul(
                                    out=ps[:nrow, r * P:(r + 1) * P],
                                    lhsT=src_c[:, wl * P: wl * P + nrow],
                                    rhs=w_t[:, r * P:(r + 1) * P],
                                    start=True, stop=True)
                            st = stp.tile([P, RELS_PER_CORE * P], f32, tag="st")
                            nc.vector.tensor_copy(out=st[:nrow, :], in_=ps[:nrow, :])
                            for r in range(RELS_PER_CORE):
                                nc.sync.dma_start(
                                    out=Hr[layer][r * N_NODES + nt0:
                                                  r * N_NODES + nt0 + nrow, :],
                                    in_=st[:nrow, r * P:(r + 1) * P])

            def scatter(layer):
                """Gather Hr rows per edge, one-hot matmul into window PSUM,
                flush per chunk into partial[layer], AR per AR-chunk."""
                with tc.tile_pool(name=f"sc{layer}", bufs=2) as slabp, \
                     tc.tile_pool(name=f"oh{layer}", bufs=12) as ohp, \
                     tc.tile_pool(name=f"pw{layer}", bufs=6, space="PSUM") as pwp, \
                     tc.tile_pool(name=f"fl{layer}", bufs=2) as flp:
                    done_ar = 0
                    for ch in range(N_CHUNKS):
                        pr = prog[ch]
                        ncol = pr["n_cols"]
                        if ncol == 0:
                            continue
                        w0, w1 = pr["w0"], pr["w1"]
                        slab = slabp.tile([P, 64 * P], f32, tag="slab")
                        sl3 = slab[:].rearrange("p (t d) -> p t d", t=64)
                        cbase = int(chunk_slab_col[ch])
                        c0 = 0
                        if SC_SUB in (36, 37, 39):
                            nc.vector.memset(slab[:, :ncol * P], 0.25)
                        for q in range(N_RANGES):
                            qc = pr["call_cols"][q]
                            if qc == 0 or SC_SUB in (36, 37, 39):
                                continue
                            done = 0
                            while done < qc:
                                piece = min(8, qc - done)
                                nidx = piece * P
                                g0 = (cbase + c0 + done) * P // 16
                                nc.gpsimd.dma_gather(
                                    out_ap=sl3[:, c0 + done:c0 + done + piece, :],
                                    in_ap=Hr[layer][q * RANGE:, :],
                                    idxs_ap=gidx_t[:, g0:g0 + nidx // 16],
                                    num_idxs=nidx,
                                    num_idxs_reg=nidx,
                                    elem_size=P,
                                )
                                done += piece
                            c0 += qc
                        c_lo = w0 * P
                        c_hi = min(w1 * P, N_NODES)
                        nfc = c_hi - c_lo
                        fl = flp.tile([P, CHUNK_W * P], f32, tag="fl")
                        pj = pr["pair_base"]
                        for (wl, lst) in (pr["wblocks"] if SC_SUB >= 3 else []):
                            pw = pwp.tile([P, P], f32, space="PSUM", tag="pw")
                            for k, cc in enumerate(lst):
                                oh = ohp.tile([P, P], f32, tag="oh")
                                nc.vector.tensor_scalar(
                                    out=oh[:],
                                    in0=iota_t[:],
                                    scalar1=drel_t[:, pj: pj + 1],
                                    scalar2=norm_t[:, pj: pj + 1],
                                    op0=mybir.AluOpType.is_equal,
                                    op1=mybir.AluOpType.mult,
                                )
                                pj += 1
                                nc.tensor.matmul(
                                    out=pw[:],
                                    lhsT=slab[:, cc * P:(cc + 1) * P],
                                    rhs=oh[:],
                                    start=(k == 0), stop=(k == len(lst) - 1),
                                )
                            nwc = min(P, nfc - wl * P)
                            nc.vector.tensor_copy(out=fl[:, wl * P: wl * P + nwc],
                                                  in_=pw[:, :nwc])
                        if SC_SUB < 4:
                            continue
                        for a, (al, ah, acl, ach) in enumerate(ar_bounds):
                            if al <= ch < ah:
                                break
                        nc.sync.dma_start(
                            out=partial[layer][a][:, c_lo - acl:c_hi - acl],
                            in_=fl[:, :nfc])
                        # fire AR when its last chunk flushed
                        if DEBUG_STAGE >= 3 and ch + 1 == ar_bounds[done_ar][1]:
                            nc.gpsimd.collective_compute(
                                "AllReduce",
                                mybir.AluOpType.add,
                                ins=[partial[layer][done_ar].ap().opt()],
                                outs=[red[layer][done_ar].ap().opt()],
                                replica_groups=[list(range(N_CORES))],
                            )
                            done_ar += 1

            if SC_SUB not in (37, 39):
                transform(0, w1_t)
            if DEBUG_STAGE >= 2:
                scatter(0)
            if DEBUG_STAGE >= 3:
                transform(1, w2_t)
                scatter(1)
            if DEBUG_STAGE >= 4:
                for a in range(len(ar_bounds)):
                    nc.sync.dma_start(out=outs[a][:], in_=red[1][a][:])
            elif DEBUG_STAGE == 2:
                with tc.tile_pool(name="dbg", bufs=2) as dbp:
                    for a, (al, ah, acl, ach) in enumerate(ar_bounds):
                        for o in range(0, ach - acl, 2048):
                            oe = min(o + 2048, ach - acl)
                            t = dbp.tile([P, 2048], f32, tag="d")
                            nc.sync.dma_start(out=t[:, :oe - o],
                                              in_=partial[0][a][:, o:oe])
                            nc.sync.dma_start(out=outs[a][:, o:oe],
                                              in_=t[:, :oe - o])
            elif DEBUG_STAGE == 3:
                with tc.tile_pool(name="dbg", bufs=2) as dbp:
                    for a, (al, ah, acl, ach) in enumerate(ar_bounds):
                        for o in range(0, ach - acl, 2048):
                            oe = min(o + 2048, ach - acl)
                            t = dbp.tile([P, 2048], f32, tag="d")
                            nc.sync.dma_start(out=t[:, :oe - o],
                                              in_=red[1][a][:, o:oe])
                            nc.sync.dma_start(out=outs[a][:, o:oe],
                                              in_=t[:, :oe - o])

    nc.compile()
    return nc


def kernel(h, norm, src, dst, etype, V1, coef1, bias1, V2, coef2, bias2):
    import hashlib
    key = hashlib.md5(
        np.asarray(src).tobytes() + np.asarray(dst).tobytes()
        + np.asarray(etype).tobytes()
    ).hexdigest() + str(DEBUG_STAGE)
    if key not in _CACHE:
        prog, drel_2d, norm_2d, gidx_2d, chunk_cols, pair_cols = _build_edge_org(
            np.asarray(src), np.asarray(dst), np.asarray(etype), np.asarray(norm))
        nc = _build_bass(prog, chunk_cols, pair_cols)
        _CACHE[key] = (nc, drel_2d, norm_2d, gidx_2d)
    nc, drel_2d, norm_2d, gidx_2d = _CACHE[key]

    W1 = np.einsum("rb,bio->rio", np.asarray(coef1), np.asarray(V1)).astype(np.float32)
    W2 = np.einsum("rb,bio->rio", np.asarray(coef2), np.asarray(V2)).astype(np.float32)
    hT = np.ascontiguousarray(np.asarray(h).T).astype(np.float32)
    iota_np = np.broadcast_to(np.arange(P, dtype=np.float32), (P, P)).copy()
    b1 = np.asarray(bias1).astype(np.float32).reshape(P, 1)

    in_maps = []
    for c in range(N_CORES):
        in_maps.append({
            "hT": hT,
            "Wp1": np.ascontiguousarray(W1[2 * c:2 * c + 2]),
            "Wp2": np.ascontiguousarray(W2[2 * c:2 * c + 2]),
            "iota": iota_np,
            "bias1": b1,
            "drel": drel_2d[c],
            "normv": norm_2d[c],
            "gidx": gidx_2d[c],
        })

    global _LAST_IN_MAPS
    _LAST_IN_MAPS = in_maps
    from concourse.bass_utils import run_bass_kernel_spmd
    res = run_bass_kernel_spmd(nc, in_maps, core_ids=list(range(N_CORES)))
    r0 = res.results[0]
    outT = np.concatenate([r0[f"out{a}"] for a in range(len(AR_CHUNK_OF))], axis=1)
    out = outT.T + np.asarray(bias2).astype(np.float32)[None, :]
    return out.astype(np.float32)


# revision 16
# speedup vs baseline: 9002513495.0000x; 3803005.0000x over previous
"""Trainium2 Bass kernel for 2-layer BaseRGCN (basis decomposition).

Strategy (8 NeuronCores, SPMD — identical program, per-core data):
  - Relation sharding: core c owns relations {2c, 2c+1}. Host folds the basis
    decomposition into per-relation weight matrices W[r] = sum_b coef[r,b] V[b]
    and hands each core its pair.
  - Per layer: transform phase computes Hr[r] = h @ W[r] for the core's two
    relations into a DRAM table [2N, d] (h^T tiles as the matmul stationary);
    scatter phase gathers Hr[rel*N + src] rows per edge with dma_gather,
    builds a one-hot selection matrix O[e, n] = norm_e * (dst_rel_e == n) on
    the vector engine, and accumulates out^T[o, n] += G^T(e,o) @ O(e,n) into
    per-dst-window PSUM slices; window groups flush once to a partial
    [128, N] (transposed) buffer.
  - Partials are AllReduce'd across the 8 cores in 4 node-range chunks so the
    collectives overlap the remaining scatter work. ReLU+bias1 is fused into
    layer 2's transform input load. bias2 + final transpose happen on host.
  - Edges are sorted by dst-window inside 16-window chunks and split into 4
    int16 index ranges (dma_gather indices are int16; table has 2N = 100000
    rows, ranges of 25000); groups are padded to 32-slot quanta and packed
    into 128-slot tiles without crossing tile boundaries.
"""

import numpy as np

N_NODES = 50000
H_DIM = 128
NUM_RELS = 16
N_CORES = 8
RELS_PER_CORE = NUM_RELS // N_CORES
P = 128                      # partitions / window size / feature dim
W_TOT = (N_NODES + P - 1) // P          # 391 dst windows (last partial: 80)
CHUNK_W = 16                 # dst windows per scatter chunk (PSUM capacity)
N_CHUNKS = (W_TOT + CHUNK_W - 1) // CHUNK_W   # 25
RANGE = 25000                # int16 index range per dma_gather call
N_RANGES = 4                 # 2*N_NODES / RANGE
QUANT = 32                   # group padding quantum (PE row-group alignment)
AR_CHUNK_OF = [6, 6, 6, 7]   # scatter chunks per all-reduce chunk
DEBUG_STAGE = 4              # 1: transform only; 2: +scatter; 3: +AR; 4: full
SC_SUB = 4                   # scatter sub-stage: 1 gathers, 2 +onehot, 3 +matmul, 4 +flush

_CACHE = {}
_LAST_IN_MAPS = None


def _build_edge_org(src, dst, etype, norm):
    """Host-side edge organization. Returns per-core arrays + uniform program
    metadata (identical across cores)."""
    rng_info = []
    # per-core edge fields
    core_edges = []
    for c in range(N_CORES):
        m = (etype // RELS_PER_CORE) == c
        s, d, e, nr = src[m], dst[m], etype[m] - c * RELS_PER_CORE, norm[m, 0]
        gidx = e.astype(np.int64) * N_NODES + s
        q = gidx // RANGE
        i16 = (gidx - q * RANGE).astype(np.int16)
        w = d // P
        drel = (d - w * P).astype(np.float32)
        core_edges.append((w, q, i16, drel, nr.astype(np.float32)))

    # unified per-(chunk, q, window) counts: max over cores (same program)
    counts = np.zeros((N_CORES, W_TOT, N_RANGES), np.int64)
    for c in range(N_CORES):
        w, q, _, _, _ = core_edges[c]
        np.add.at(counts[c], (w, q), 1)
    mx = counts.max(axis=0)                       # [W_TOT, N_RANGES]
    mx[:, 0] = np.maximum(mx[:, 0], 1)            # every window gets >=1 slot

    # tight packing: per (chunk, q): windows back-to-back, call padded to 128.
    # matmuls are full-128 per (tile-col, window) pair with window-masked
    # one-hots (no sub-partition matmuls - HW accumulation-group bug).
    prog = []
    chunk_cols = []
    slot_of = []        # per chunk: (q, w) -> chunk-local slot start
    pair_cols = 0
    for ch in range(N_CHUNKS):
        w0 = ch * CHUNK_W
        w1 = min(w0 + CHUNK_W, W_TOT)
        gslot = {}
        call_cols = []
        col_base = 0
        pairs = []        # (col_local, w_local) in slot order
        for q in range(N_RANGES):
            pos = 0
            for w in range(w0, w1):
                cnt = int(mx[w, q])
                if cnt == 0:
                    continue
                gslot[(q, w)] = col_base * P + pos
                c_lo = col_base + pos // P
                c_hi = col_base + (pos + cnt - 1) // P
                for cc in range(c_lo, c_hi + 1):
                    pairs.append((cc, w - w0))
                pos += cnt
            qc = (pos + P - 1) // P
            call_cols.append(qc)
            col_base += qc
        # window-major pair ordering with first/last flags
        win_pairs = {}
        for (cc, wl) in pairs:
            win_pairs.setdefault(wl, []).append(cc)
        wblocks = []
        for wl in sorted(win_pairs):
            lst = sorted(set(win_pairs[wl]))
            wblocks.append((wl, lst))
        n_pairs = sum(len(lst) for _, lst in wblocks)
        prog.append(dict(call_cols=call_cols, wblocks=wblocks,
                         n_cols=col_base, n_pairs=n_pairs, w0=w0, w1=w1,
                         pair_base=pair_cols))
        slot_of.append(gslot)
        chunk_cols.append(col_base)
        pair_cols += n_pairs
        assert col_base <= 64, f"chunk {ch}: {col_base} cols"

    tot_cols = int(np.sum(chunk_cols))
    chunk_slab_off = np.concatenate([[0], np.cumsum(chunk_cols)]) * P

    # per-core slot arrays
    idx16 = np.zeros((N_CORES, tot_cols * P), np.int16)
    slot_w = np.full(tot_cols * P, -1, np.int64)       # slot -> global window
    drel_slot = np.full((N_CORES, tot_cols * P), -1.0, np.float32)
    norm_slot = np.zeros((N_CORES, tot_cols * P), np.float32)
    for c in range(N_CORES):
        w, q, i16, drel, nr = core_edges[c]
        ch = w // CHUNK_W
        order = np.lexsort((w, q, ch))
        cursor = {}
        for ei in order:
            key = (int(ch[ei]), int(q[ei]), int(w[ei]))
            ofs = cursor.get(key, 0)
            cursor[key] = ofs + 1
            slot = int(chunk_slab_off[key[0]]) + slot_of[key[0]][(key[1], key[2])] + ofs
            idx16[c, slot] = i16[ei]
            drel_slot[c, slot] = drel[ei]
            norm_slot[c, slot] = nr[ei]
            slot_w[slot] = key[2]

    # per-(pair) dstrel/norm columns: window-masked
    drel_2d = np.full((N_CORES, P, pair_cols), -1.0, np.float32)
    norm_2d = np.zeros((N_CORES, P, pair_cols), np.float32)
    pc = 0
    for ch in range(N_CHUNKS):
        pr = prog[ch]
        cbase = int(chunk_slab_off[ch]) // P
        pair_index = {}
        for (wl, lst) in pr["wblocks"]:
            for cc in lst:
                pair_index[(cc, wl)] = pc
                pc += 1
        for (cc, wl), pj in pair_index.items():
            s0 = (cbase + cc) * P
            wglob = pr["w0"] + wl
            for c in range(N_CORES):
                mask = slot_w[s0:s0 + P] == wglob
                drel_2d[c, mask, pj] = drel_slot[c, s0:s0 + P][mask]
                norm_2d[c, mask, pj] = norm_slot[c, s0:s0 + P][mask]
    assert pc == pair_cols

    # dma_gather idx arrays (wrap 16 + replicate to 128)
    gidx_2d = np.zeros((N_CORES, 128, tot_cols * P // 16), np.int16)
    for c in range(N_CORES):
        a = idx16[c].reshape(tot_cols * P // 16, 16).T
        gidx_2d[c] = np.tile(a, (8, 1))
    return prog, drel_2d, norm_2d, gidx_2d, chunk_cols, pair_cols


def _build_bass(prog, chunk_cols, pair_cols):
    from concourse import bacc, bass, mybir, tile

    f32 = mybir.dt.float32
    i16 = mybir.dt.int16
    nc = bacc.Bacc("TRN2", target_bir_lowering=False, debug=False,
                   num_devices=N_CORES, dynamic_dma_scratch_size=65536)

    tot_cols = int(np.sum(chunk_cols))
    hT = nc.dram_tensor("hT", [P, N_NODES], f32, kind="ExternalInput")
    Wp1 = nc.dram_tensor("Wp1", [RELS_PER_CORE, P, P], f32, kind="ExternalInput")
    Wp2 = nc.dram_tensor("Wp2", [RELS_PER_CORE, P, P], f32, kind="ExternalInput")
    iota = nc.dram_tensor("iota", [P, P], f32, kind="ExternalInput")
    bias1 = nc.dram_tensor("bias1", [P, 1], f32, kind="ExternalInput")
    drel = nc.dram_tensor("drel", [P, pair_cols], f32, kind="ExternalInput")
    normv = nc.dram_tensor("normv", [P, pair_cols], f32, kind="ExternalInput")
    gidx = nc.dram_tensor("gidx", [P, tot_cols * 8], i16, kind="ExternalInput")

    # AR chunk column extents
    ar_bounds = []
    chs = 0
    for nch in AR_CHUNK_OF:
        ch_lo, ch_hi = chs, chs + nch
        w_lo = ch_lo * CHUNK_W
        w_hi = min(ch_hi * CHUNK_W, W_TOT)
        c_lo = w_lo * P
        c_hi = min(w_hi * P, N_NODES)
        ar_bounds.append((ch_lo, ch_hi, c_lo, c_hi))
        chs = ch_hi

    bf16 = mybir.dt.bfloat16
    Hr = [nc.dram_tensor(f"Hr{l}", [2 * N_NODES, P], bf16) for l in (0, 1)]
    partial = [[nc.dram_tensor(f"pt{l}_{a}", [P, b[3] - b[2]], f32)
                for a, b in enumerate(ar_bounds)] for l in (0, 1)]
    red = [[nc.dram_tensor(f"rd{l}_{a}", [P, b[3] - b[2]], f32,
                           addr_space="Shared")
            for a, b in enumerate(ar_bounds)] for l in (0, 1)]
    outs = [nc.dram_tensor(f"out{a}", [P, b[3] - b[2]], f32,
                           kind="ExternalOutput")
            for a, b in enumerate(ar_bounds)]

    chunk_slab_col = np.concatenate([[0], np.cumsum(chunk_cols)]).astype(int)

    with tile.TileContext(nc) as tc:
        with tc.tile_pool(name="const", bufs=1) as constp:
            iota_t = constp.tile([P, P], f32)
            nc.sync.dma_start(out=iota_t[:], in_=iota[:])
            bias1_t = constp.tile([P, 1], f32)
            nc.sync.dma_start(out=bias1_t[:], in_=bias1[:])
            drel_t = constp.tile([P, pair_cols], f32)
            nc.sync.dma_start(out=drel_t[:], in_=drel[:])
            norm_t = constp.tile([P, pair_cols], f32)
            nc.sync.dma_start(out=norm_t[:], in_=normv[:])
            gidx_t = constp.tile([P, tot_cols * 8], i16)
            nc.sync.dma_start(out=gidx_t[:], in_=gidx[:])
            w1_t = constp.tile([P, RELS_PER_CORE * P], f32)
            for r in range(RELS_PER_CORE):
                nc.sync.dma_start(out=w1_t[:, r * P:(r + 1) * P], in_=Wp1[r])
            w2_t = constp.tile([P, RELS_PER_CORE * P], f32)
            for r in range(RELS_PER_CORE):
                nc.sync.dma_start(out=w2_t[:, r * P:(r + 1) * P], in_=Wp2[r])

            def transform(layer, w_t):
                """Compute Hr[layer] from h^T source. layer 0: hT input;
                layer 1: relu(red[0] + bias1)."""
                with tc.tile_pool(name=f"tf{layer}", bufs=3) as sb, \
                     tc.tile_pool(name=f"tfp{layer}", bufs=4, space="PSUM") as pp, \
                     tc.tile_pool(name=f"tfs{layer}", bufs=3) as stp:
                    for ch in range(N_CHUNKS):
                        w0 = ch * CHUNK_W
                        w1 = min(w0 + CHUNK_W, W_TOT)
                        c_lo = w0 * P
                        c_hi = min(w1 * P, N_NODES)
                        ncols = c_hi - c_lo
                        ht_c = sb.tile([P, CHUNK_W * P], f32, tag="ht")
                        if layer == 0:
                            nc.sync.dma_start(out=ht_c[:, :ncols],
                                              in_=hT[:, c_lo:c_hi])
                            src_c = ht_c
                        else:
                            # find AR chunk containing this scatter chunk
                            for a, (al, ah, acl, ach) in enumerate(ar_bounds):
                                if al <= ch < ah:
                                    break
                            nc.sync.dma_start(
                                out=ht_c[:, :ncols],
                                in_=red[0][a][:, c_lo - acl:c_hi - acl])
                            act_c = sb.tile([P, CHUNK_W * P], f32, tag="act")
                            nc.scalar.activation(
                                out=act_c[:, :ncols], in_=ht_c[:, :ncols],
                                func=mybir.ActivationFunctionType.Relu,
                                bias=bias1_t[:, :1], scale=1.0)
                            src_c = act_c
                        for wl in range(w1 - w0):
                            nt0 = (w0 + wl) * P
                            nrow = min(P, N_NODES - nt0)
                            ps = pp.tile([P, RELS_PER_CORE * P], f32,
                                         space="PSUM", tag="ps")
                            for r in range(RELS_PER_CORE):
                                nc.tensor.matmul(
                                    out=ps[:nrow, r * P:(r + 1) * P],
                                    lhsT=src_c[:, wl * P: wl * P + nrow],
                                    rhs=w_t[:, r * P:(r + 1) * P],
                                    start=True, stop=True)
                            st = stp.tile([P, RELS_PER_CORE * P], bf16, tag="st")
                            nc.vector.tensor_copy(out=st[:nrow, :], in_=ps[:nrow, :])
                            for r in range(RELS_PER_CORE):
                                nc.sync.dma_start(
                                    out=Hr[layer][r * N_NODES + nt0:
                                                  r * N_NODES + nt0 + nrow, :],
                                    in_=st[:nrow, r * P:(r + 1) * P])

            def scatter(layer):
                """Gather Hr rows per edge, one-hot matmul into window PSUM,
                flush per chunk into partial[layer], AR per AR-chunk."""
                with tc.tile_pool(name=f"sc{layer}", bufs=2) as slabp, \
                     tc.tile_pool(name=f"oh{layer}", bufs=12) as ohp, \
                     tc.tile_pool(name=f"pw{layer}", bufs=6, space="PSUM") as pwp, \
                     tc.tile_pool(name=f"fl{layer}", bufs=2) as flp:
                    done_ar = 0
                    for ch in range(N_CHUNKS):
                        pr = prog[ch]
                        ncol = pr["n_cols"]
                        if ncol == 0:
                            continue
                        w0, w1 = pr["w0"], pr["w1"]
                        slab = slabp.tile([P, 64 * P], bf16, tag="slab")
                        sl3 = slab[:].rearrange("p (t d) -> p t d", t=64)
                        cbase = int(chunk_slab_col[ch])
                        c0 = 0
                        if SC_SUB in (36, 37, 39):
                            nc.vector.memset(slab[:, :ncol * P], 0.25)
                        for q in range(N_RANGES):
                            qc = pr["call_cols"][q]
                            if qc == 0 or SC_SUB in (36, 37, 39):
                                continue
                            done = 0
                            while done < qc:
                                piece = min(8, qc - done)
                                nidx = piece * P
                                g0 = (cbase + c0 + done) * P // 16
                                nc.gpsimd.dma_gather(
                                    out_ap=sl3[:, c0 + done:c0 + done + piece, :],
                                    in_ap=Hr[layer][q * RANGE:, :],
                                    idxs_ap=gidx_t[:, g0:g0 + nidx // 16],
                                    num_idxs=nidx,
                                    num_idxs_reg=nidx,
                                    elem_size=P,
                                )
                                done += piece
                            c0 += qc
                        c_lo = w0 * P
                        c_hi = min(w1 * P, N_NODES)
                        nfc = c_hi - c_lo
                        fl = flp.tile([P, CHUNK_W * P], f32, tag="fl")
                        pj = pr["pair_base"]
                        for (wl, lst) in (pr["wblocks"] if SC_SUB >= 3 else []):
                            pw = pwp.tile([P, P], f32, space="PSUM", tag="pw")
                            for k, cc in enumerate(lst):
                                oh = ohp.tile([P, P], bf16, tag="oh")
                                nc.vector.tensor_scalar(
                                    out=oh[:],
                                    in0=iota_t[:],
                                    scalar1=drel_t[:, pj: pj + 1],
                                    scalar2=norm_t[:, pj: pj + 1],
                                    op0=mybir.AluOpType.is_equal,
                                    op1=mybir.AluOpType.mult,
                                )
                                pj += 1
                                nc.tensor.matmul(
                                    out=pw[:],
                                    lhsT=slab[:, cc * P:(cc + 1) * P],
                                    rhs=oh[:],
                                    start=(k == 0), stop=(k == len(lst) - 1),
                                )
                            nwc = min(P, nfc - wl * P)
                            nc.vector.tensor_copy(out=fl[:, wl * P: wl * P + nwc],
                                                  in_=pw[:, :nwc])
                        if SC_SUB < 4:
                            continue
                        for a, (al, ah, acl, ach) in enumerate(ar_bounds):
                            if al <= ch < ah:
                                break
                        nc.sync.dma_start(
                            out=partial[layer][a][:, c_lo - acl:c_hi - acl],
                            in_=fl[:, :nfc])
                        # fire AR when its last chunk flushed
                        if DEBUG_STAGE >= 3 and ch + 1 == ar_bounds[done_ar][1]:
                            nc.gpsimd.collective_compute(
                                "AllReduce",
                                mybir.AluOpType.add,
                                ins=[partial[layer][done_ar].ap().opt()],
                                outs=[red[layer][done_ar].ap().opt()],
                                replica_groups=[list(range(N_CORES))],
                            )
                            done_ar += 1

            if SC_SUB not in (37, 39):
                transform(0, w1_t)
            if DEBUG_STAGE >= 2:
                scatter(0)
            if DEBUG_STAGE >= 3:
                transform(1, w2_t)
                scatter(1)
            if DEBUG_STAGE >= 4:
                for a in range(len(ar_bounds)):
                    nc.sync.dma_start(out=outs[a][:], in_=red[1][a][:])
            elif DEBUG_STAGE == 2:
                with tc.tile_pool(name="dbg", bufs=2) as dbp:
                    for a, (al, ah, acl, ach) in enumerate(ar_bounds):
                        for o in range(0, ach - acl, 2048):
                            oe = min(o + 2048, ach - acl)
                            t = dbp.tile([P, 2048], f32, tag="d")
                            nc.sync.dma_start(out=t[:, :oe - o],
                                              in_=partial[0][a][:, o:oe])
                            nc.sync.dma_start(out=outs[a][:, o:oe],
                                              in_=t[:, :oe - o])
            elif DEBUG_STAGE == 3:
                with tc.tile_pool(name="dbg", bufs=2) as dbp:
                    for a, (al, ah, acl, ach) in enumerate(ar_bounds):
                        for o in range(0, ach - acl, 2048):
                            oe = min(o + 2048, ach - acl)
                            t = dbp.tile([P, 2048], f32, tag="d")
                            nc.sync.dma_start(out=t[:, :oe - o],
                                              in_=red[1][a][:, o:oe])
                            nc.sync.dma_start(out=outs[a][:, o:oe],
                                              in_=t[:, :oe - o])

    nc.compile()
    return nc


def kernel(h, norm, src, dst, etype, V1, coef1, bias1, V2, coef2, bias2):
    import hashlib
    key = hashlib.md5(
        np.asarray(src).tobytes() + np.asarray(dst).tobytes()
        + np.asarray(etype).tobytes()
    ).hexdigest() + str(DEBUG_STAGE)
    if key not in _CACHE:
        prog, drel_2d, norm_2d, gidx_2d, chunk_cols, pair_cols = _build_edge_org(
            np.asarray(src), np.asarray(dst), np.asarray(etype), np.asarray(norm))
        nc = _build_bass(prog, chunk_cols, pair_cols)
        _CACHE[key] = (nc, drel_2d, norm_2d, gidx_2d)
    nc, drel_2d, norm_2d, gidx_2d = _CACHE[key]

    W1 = np.einsum("rb,bio->rio", np.asarray(coef1), np.asarray(V1)).astype(np.float32)
    W2 = np.einsum("rb,bio->rio", np.asarray(coef2), np.asarray(V2)).astype(np.float32)
    hT = np.ascontiguousarray(np.asarray(h).T).astype(np.float32)
    iota_np = np.broadcast_to(np.arange(P, dtype=np.float32), (P, P)).copy()
    b1 = np.asarray(bias1).astype(np.float32).reshape(P, 1)

    in_maps = []
    for c in range(N_CORES):
        in_maps.append({
            "hT": hT,
            "Wp1": np.ascontiguousarray(W1[2 * c:2 * c + 2]),
            "Wp2": np.ascontiguousarray(W2[2 * c:2 * c + 2]),
            "iota": iota_np,
            "bias1": b1,
            "drel": drel_2d[c],
            "normv": norm_2d[c],
            "gidx": gidx_2d[c],
        })

    global _LAST_IN_MAPS
    _LAST_IN_MAPS = in_maps
    from concourse.bass_utils import run_bass_kernel_spmd
    res = run_bass_kernel_spmd(nc, in_maps, core_ids=list(range(N_CORES)))
    r0 = res.results[0]
    outT = np.concatenate([r0[f"out{a}"] for a in range(len(AR_CHUNK_OF))], axis=1)
    out = outT.T + np.asarray(bias2).astype(np.float32)[None, :]
    return out.astype(np.float32)
